# revision 9
# baseline (speedup 1.0000x reference)
"""Trainium2 Bass kernel for CausalAttention (sliding-window + scale-frame sparse attention).

Problem shape (hardcoded): B=1, N=4096, C=512, H=8, Dh=64, frame_seqlen=256,
sliding_window_size=2, num_frame_per_block=1, num_frame_for_scale=2.

Sharding: sequence-parallel over 8 NeuronCores. Core i owns queries
[512*i, 512*(i+1)) (= frames 2i, 2i+1) and returns that slice of the final
output. Keys per core: 512 "scale" tokens (frames 0,1; attended by every query
unconditionally) plus a 3-frame window {2i-1, 2i, 2i+1}. No collectives.

All matmuls are bf16 with fp32 PSUM accumulation (fp8 was measured to break
the 2e-2 gate: this data has peaked attention queries where per-element fp8
noise lands directly in the max-err metric). Structure (v2):
  - Block-validity masking is folded into the data: window-key columns of x
    are zeroed host-side for dead frames (queries read a dedicated column
    range), and the per-k-tile ones-flag column of V is 0 for dead frames, so
    dead tiles contribute nothing to numerator or denominator. Only the pure
    tril mask remains on-device (4 [128,128] multiplies per head).
  - Score PSUM uses 2-bank (1024-col) merged tiles so each head needs only 4
    exp instructions; masked sub-blocks of the window are never scored
    (kt7/kt8/kt9 get trimmed query ranges).
  - Row 64 of the A-V accumulator collects softmax sums via the flag column.
  - Output projection packs head pairs on 128 partitions (16 matmuls); odd
    heads land at partitions 64:128 via a partition-shifting SBUF-SBUF DMA.
  - K bias is dropped (adds a per-query constant to scores: softmax
    invariant); V bias folds into the proj bias host-side.
  - PE warms up on a tiny tile that arrives in the first DMA.
"""

from contextlib import ExitStack

import numpy as np
import ml_dtypes

N, C, H, DH = 4096, 512, 8, 64
F = 256                 # frame_seqlen
NCORES = 8
NQ = N // NCORES        # 512 queries per core (2 frames)
KS = 512                # scale tokens (frames 0,1)
KW = 3 * F              # window tokens per core
NK = KS + KW            # 1280 key/value tokens per core
NKX = NK + NQ           # + dedicated query columns
BF16 = ml_dtypes.bfloat16
NWARM = 36

_CACHE = {}


def _build(repeat=1):
    """Build + compile the (single, SPMD) Bass program. Returns nc."""
    import concourse.bass as bass  # noqa: F401
    import concourse.mybir as mybir
    import concourse.tile as tile
    from concourse import bacc

    f32 = mybir.dt.float32
    bf16 = mybir.dt.bfloat16
    EXP = mybir.ActivationFunctionType.Exp

    nc = bacc.Bacc("TRN2", target_bir_lowering=False, debug=False)

    xT = nc.dram_tensor("xT", [128, 4 * NKX], bf16, kind="ExternalInput")
    wqT = nc.dram_tensor("wqT", [128, 4 * C], bf16, kind="ExternalInput")
    wkT = nc.dram_tensor("wkT", [128, 4 * C], bf16, kind="ExternalInput")
    wvT = nc.dram_tensor("wvT", [128, 4 * C], bf16, kind="ExternalInput")
    wp2 = nc.dram_tensor("wp2", [128, 4 * C], bf16, kind="ExternalInput")
    btab = nc.dram_tensor("btab", [128, 8], f32, kind="ExternalInput")
    trilm = nc.dram_tensor("trilm", [128, 128], bf16, kind="ExternalInput")
    vfl = nc.dram_tensor("vfl", [128, 80], bf16, kind="ExternalInput")
    outT = nc.dram_tensor("outT", [C, NQ], f32, kind="ExternalOutput")

    SC = float(DH) ** -0.5

    with tile.TileContext(nc) as tc, ExitStack() as ctx:
        cp = ctx.enter_context(tc.tile_pool(name="const", bufs=1))
        dp = ctx.enter_context(tc.tile_pool(name="data", bufs=1))
        ptp = ctx.enter_context(tc.tile_pool(name="pt", bufs=8))
        recp = ctx.enter_context(tc.tile_pool(name="rec", bufs=2))
        ppp = ctx.enter_context(tc.tile_pool(name="pp", bufs=2, space="PSUM"))
        stp = ctx.enter_context(tc.tile_pool(name="st", bufs=3, space="PSUM"))

        def body():
            # ---- input DMAs (tm first: it feeds the PE warmup) ----
            tm = cp.tile([128, 128], bf16, tag="tm")
            nc.sync.dma_start(tm[:], trilm.ap())
            bt = cp.tile([128, 8], f32, tag="bt")
            nc.sync.dma_start(bt[:], btab.ap())
            xs = cp.tile([128, 4, NKX], bf16, tag="xs")
            xr = xT.ap().rearrange("p (a t) -> p a t", a=4)
            wv = cp.tile([128, 4, C], bf16, tag="wv")
            nc.sync.dma_start(wv[:], wvT.ap().rearrange("p (a o) -> p a o", a=4))
            for t0, t1 in ((0, 448), (448, 896), (896, 1280), (1280, NKX)):
                nc.sync.dma_start(xs[:, :, t0:t1], xr[:, :, t0:t1])
            wk = cp.tile([128, 4, C], bf16, tag="wk")
            nc.sync.dma_start(wk[:], wkT.ap().rearrange("p (a o) -> p a o", a=4))
            wq = cp.tile([128, 4, C], bf16, tag="wq")
            nc.sync.dma_start(wq[:], wqT.ap().rearrange("p (a o) -> p a o", a=4))
            wp = cp.tile([128, 4, C], bf16, tag="wp")
            nc.sync.dma_start(wp[:], wp2.ap().rearrange("p (a o) -> p a o", a=4))

            V = dp.tile([128, 10, H, DH + 1], bf16, tag="V")
            # flags land in a compact tile; the strided column write goes via
            # DVE (2-byte strided DMA scatter races with the value copies)
            vflt = cp.tile([128, 80], bf16, tag="vflt")
            nc.sync.dma_start(vflt[:], vfl.ap())
            nc.vector.tensor_copy(
                V[:, :, :, DH:DH + 1],
                vflt.rearrange("p (a h u) -> p a h u", a=10, h=H, u=1))

            # warmup: ramp the PE p-state while the big DMAs land
            wps = ppp.tile([128, 512], f32, tag="pp", name="wps")
            for _ in range(NWARM):
                nc.tensor.matmul(wps[:, 0:128], lhsT=tm[:], rhs=tm[:],
                                 start=True, stop=True, skip_group_check=True)
            # preload the exp table during the DMA window
            dumt = cp.tile([1, 16], bf16, tag="dumt")
            nc.scalar.activation(dumt[:], tm[0:1, 0:16], EXP, scale=1.0)

            QT = dp.tile([128, 4, NQ], bf16, tag="QT")
            KT = dp.tile([128, 4, NK], bf16, tag="KT")
            OT = dp.tile([128, 4, NQ], bf16, tag="OT")
            oT = dp.tile([128, 4, NQ], f32, tag="oT")

            def qkv_psum(n):
                if n % 2 == 0:
                    t = ppp.tile([128, 512], f32, tag="pp", name="qps")
                    return t[:, 0:512]
                t = stp.tile([128, 1024], f32, tag="st", name="qps")
                return t[:, 0:512]

            # ---- V projection (natural layout) ----
            for tt in range(10):
                ps = qkv_psum(tt)
                for ci in range(4):
                    nc.tensor.matmul(ps[:], lhsT=xs[:, ci, 128 * tt:128 * (tt + 1)],
                                     rhs=wv[:, ci, :], start=(ci == 0),
                                     stop=(ci == 3))
                nc.vector.tensor_copy(V[:, tt, :, 0:DH],
                                      ps.rearrange("p (h d) -> p h d", h=H))

            # ---- per-pair Q/K projection interleaved with attention ----
            for p in range(4):
                ps = qkv_psum(p)
                for ci in range(4):
                    nc.tensor.matmul(ps[:], lhsT=wq[:, ci, 128 * p:128 * (p + 1)],
                                     rhs=xs[:, ci, NK:NKX], start=(ci == 0),
                                     stop=(ci == 3))
                nc.vector.tensor_scalar_add(QT[:, p, :], ps[:], bt[:, p:p + 1])
                kt2b = stp.tile([128, 1024], f32, tag="st", name="kps")
                for half in range(2):
                    for ci in range(4):
                        nc.tensor.matmul(kt2b[:, 512 * half:512 * (half + 1)],
                                         lhsT=wk[:, ci, 128 * p:128 * (p + 1)],
                                         rhs=xs[:, ci, 512 * half:512 * (half + 1)],
                                         start=(ci == 0), stop=(ci == 3))
                nc.vector.tensor_copy(KT[:, p, 0:1024], kt2b[:])
                ps3 = qkv_psum(p + 1)
                for ci in range(4):
                    nc.tensor.matmul(ps3[:, 0:256],
                                     lhsT=wk[:, ci, 128 * p:128 * (p + 1)],
                                     rhs=xs[:, ci, 1024:1280], start=(ci == 0),
                                     stop=(ci == 3))
                nc.vector.tensor_copy(KT[:, p, 1024:1280], ps3[:, 0:256])

                for h in (2 * p, 2 * p + 1):
                    par = h % 2
                    prow = slice(par * 64, par * 64 + 64)
                    qh = QT[prow, p, :]

                    def kk(kt):
                        return KT[prow, p, 128 * kt:128 * (kt + 1)]

                    # scale scores -> exp (two 2-bank tiles, no mask needed)
                    pts = []
                    for g in range(2):
                        st = stp.tile([128, 1024], f32, tag="st", name="st")
                        nc.tensor.matmul(st[:, 0:512], lhsT=kk(2 * g), rhs=qh,
                                         start=True, stop=True)
                        nc.tensor.matmul(st[:, 512:1024], lhsT=kk(2 * g + 1),
                                         rhs=qh, start=True, stop=True)
                        pt = ptp.tile([128, 1024], bf16, tag="pt", name="pt")
                        nc.scalar.activation(pt[:], st[:], EXP, scale=SC)
                        pts.append(pt)
                    # win0: [kt6 (q0:512) | kt7 (q128:512) | pad]
                    w0 = stp.tile([128, 1024], f32, tag="st", name="w0")
                    nc.tensor.matmul(w0[:, 0:512], lhsT=kk(6), rhs=qh,
                                     start=True, stop=True)
                    nc.tensor.matmul(w0[:, 512:896], lhsT=kk(7), rhs=qh[:, 128:512],
                                     start=True, stop=True)
                    pw0 = ptp.tile([128, 1024], bf16, tag="pt", name="pw0")
                    nc.scalar.activation(pw0[:, 0:896], w0[:, 0:896], EXP, scale=SC)
                    # win1: [kt4 q0:256 | kt5 q0:256 | kt9 q384:512 | kt8 q256:512]
                    w1 = stp.tile([128, 1024], f32, tag="st", name="w1")
                    nc.tensor.matmul(w1[:, 0:256], lhsT=kk(4), rhs=qh[:, 0:256],
                                     start=True, stop=True, skip_group_check=True)
                    nc.tensor.matmul(w1[:, 256:512], lhsT=kk(5), rhs=qh[:, 0:256],
                                     start=True, stop=True, skip_group_check=True)
                    nc.tensor.matmul(w1[:, 512:640], lhsT=kk(9), rhs=qh[:, 384:512],
                                     start=True, stop=True, skip_group_check=True)
                    nc.tensor.matmul(w1[:, 640:896], lhsT=kk(8), rhs=qh[:, 256:512],
                                     start=True, stop=True, skip_group_check=True)
                    pw1 = ptp.tile([128, 1024], bf16, tag="pt", name="pw1")
                    nc.scalar.activation(pw1[:, 0:896], w1[:, 0:896], EXP, scale=SC)
                    # pure-tril masks (block validity is folded into x / V flags)
                    nc.vector.tensor_mul(pw0[:, 0:128], pw0[:, 0:128], tm[:])
                    nc.vector.tensor_mul(pw0[:, 512:640], pw0[:, 512:640], tm[:])
                    nc.gpsimd.tensor_mul(pw1[:, 512:640], pw1[:, 512:640], tm[:])
                    nc.gpsimd.tensor_mul(pw1[:, 640:768], pw1[:, 640:768], tm[:])

                    # A-V accumulation; row 64 collects softmax sums via the
                    # flag column.
                    av = ppp.tile([128, 512], f32, tag="pp", name="av")
                    avs = av[0:65, :]

                    def avmm(kt, rhs, cols, start=False, stop=False):
                        nc.tensor.matmul(avs[:, cols[0]:cols[1]],
                                         lhsT=V[:, kt, h, :], rhs=rhs,
                                         start=start, stop=stop,
                                         skip_group_check=True)

                    avmm(0, pts[0][:, 0:512], (0, 512), start=True)
                    avmm(1, pts[0][:, 512:1024], (0, 512))
                    avmm(2, pts[1][:, 0:512], (0, 512))
                    avmm(3, pts[1][:, 512:1024], (0, 512))
                    avmm(6, pw0[:, 0:512], (0, 512))
                    avmm(7, pw0[:, 512:896], (128, 512))
                    avmm(4, pw1[:, 0:256], (0, 256))
                    avmm(5, pw1[:, 256:512], (0, 256))
                    avmm(9, pw1[:, 512:640], (384, 512))
                    avmm(8, pw1[:, 640:896], (256, 512), stop=True)

                    # normalize; odd heads shift to partitions 64:128 via DMA
                    sm = recp.tile([128, NQ], f32, tag="sm", name="sm")
                    nc.vector.tensor_copy(sm[0:65, :], avs[:])
                    rs = recp.tile([128, 4], f32, tag="rs", name="rs")
                    nc.sync.dma_start(rs[:], sm[64:65, :])
                    nc.vector.reciprocal(rs[:], rs[:])
                    # NOTE: partition_broadcast on HW always sources partition
                    # 0 of the tile (the in_ap offset is ignored), so seed row
                    # 0 and broadcast all 128 rows.
                    rcb = recp.tile([128, NQ], f32, tag="rcb", name="rcb")
                    nc.sync.dma_start(rcb[0:1, :], rs[:])
                    nc.gpsimd.partition_broadcast(rcb[:, :], rcb[0:1, :])
                    if par == 0:
                        nc.vector.tensor_mul(OT[0:64, p, :], sm[0:64, :],
                                             rcb[0:64, :])
                    else:
                        sv = recp.tile([128, NQ], f32, tag="sv", name="sv")
                        nc.sync.dma_start(sv[64:128, :], sm[0:64, :])
                        nc.vector.tensor_mul(OT[64:128, p, :], sv[64:128, :],
                                             rcb[64:128, :])

            # ---- output projection (head pairs on 128 partitions) ----
            od = outT.ap().rearrange("(a p) q -> p a q", p=128)
            pjs = {}

            def proj_pair(ot, jp):
                nc.tensor.matmul(pjs[ot][:], lhsT=wp[:, jp, 128 * ot:128 * (ot + 1)],
                                 rhs=OT[:, jp, :], start=(jp == 0), stop=(jp == 3),
                                 skip_group_check=True)

            def proj_out(ot):
                proj_pair(ot, 3)
                nc.vector.tensor_scalar_add(oT[:, ot, :], pjs[ot][:],
                                            bt[:, 4 + ot:5 + ot])
                nc.sync.dma_start(od[:, ot, :], oT[:, ot, :])

            for ot in (0, 1):
                pjs[ot] = ppp.tile([128, 512], f32, tag="pp", name="pj")
                for jp in range(3):
                    proj_pair(ot, jp)
            proj_out(0)
            pjs[2] = ppp.tile([128, 512], f32, tag="pp", name="pj")
            for jp in range(3):
                proj_pair(2, jp)
            proj_out(1)
            pjs[3] = ppp.tile([128, 512], f32, tag="pp", name="pj")
            for jp in range(3):
                proj_pair(3, jp)
            proj_out(2)
            proj_out(3)

        if repeat == 1:
            body()
        else:
            with tc.For_i(0, repeat, 1):
                body()

    nc.compile()
    return nc


def _get_nc(repeat=1):
    key = ("nc", repeat)
    if key not in _CACHE:
        _CACHE[key] = _build(repeat)
    return _CACHE[key]


def _host_prep(x, qkv_w, qkv_b, proj_w, proj_b):
    """Build the 8 per-core input maps."""
    x = np.asarray(x, np.float32).reshape(N, C)
    qkv_w = np.asarray(qkv_w, np.float32)
    qkv_b = np.asarray(qkv_b, np.float32)
    proj_w = np.asarray(proj_w, np.float32)
    proj_b = np.asarray(proj_b, np.float32)

    def chunk_w(w):  # w: [C_in, C_out] -> [128, 4*C_out], 128-row chunks
        a = w.reshape(4, 128, -1)              # [ci, p, o]
        return np.ascontiguousarray(
            a.transpose(1, 0, 2).reshape(128, -1).astype(BF16))

    wqT = chunk_w(qkv_w[0:C].T)
    wkT = chunk_w(qkv_w[C:2 * C].T)
    wvT = chunk_w(qkv_w[2 * C:3 * C].T)

    # proj weight packed per head pair: wp2[p, jp, o] = proj_w[o, h*64 + p%64],
    # h = 2*jp + p//64
    wpr = proj_w.T.reshape(4, 2, 64, C)        # [jp, hpar, dh, o]
    wp2 = np.ascontiguousarray(
        wpr.transpose(1, 2, 0, 3).reshape(128, 4 * C).astype(BF16))

    pb_eff = proj_b + qkv_b[2 * C:3 * C] @ proj_w.T
    btab = np.zeros((128, 8), np.float32)
    for ot in range(4):
        btab[:, ot] = qkv_b[0:C][128 * ot:128 * (ot + 1)]
        btab[:, 4 + ot] = pb_eff[128 * ot:128 * (ot + 1)]

    tril01 = (np.arange(128)[:, None] <= np.arange(128)[None, :])
    trilm = np.ascontiguousarray(tril01.astype(BF16))

    in_maps = []
    for i in range(NCORES):
        frames = (2 * i - 1, 2 * i, 2 * i + 1)
        live = [f >= 2 for f in frames]
        xc = np.zeros((NKX, C), np.float32)
        xc[0:KS] = x[0:KS]
        for w, (fr, lv) in enumerate(zip(frames, live)):
            if lv:
                xc[KS + F * w:KS + F * (w + 1)] = x[F * fr:F * (fr + 1)]
        xc[NK:NKX] = x[NQ * i:NQ * (i + 1)]
        xa = xc.T.reshape(4, 128, NKX)          # [ci, p, t]
        xTa = np.ascontiguousarray(
            xa.transpose(1, 0, 2).reshape(128, 4 * NKX).astype(BF16))

        # ones-flag column per (kt, h): 1 if the k-tile's frame is live
        kt_live = [True] * 4 + [live[0]] * 2 + [live[1]] * 2 + [live[2]] * 2
        vfla = np.zeros((128, 10, H), np.float32)
        for kt in range(10):
            if kt_live[kt]:
                vfla[:, kt, :] = 1.0
        vfla = np.ascontiguousarray(vfla.reshape(128, 80).astype(BF16))

        in_maps.append({
            "xT": xTa, "wqT": wqT, "wkT": wkT, "wvT": wvT, "wp2": wp2,
            "btab": btab, "trilm": trilm, "vfl": vfla,
        })
    return in_maps


def _check_fixed_params(block_mask, video_mask, frame_seqlen,
                        sliding_window_size, num_frame_per_block,
                        num_frame_for_scale):
    if int(frame_seqlen) != F or int(sliding_window_size) != 2 \
            or int(num_frame_per_block) != 1 or int(num_frame_for_scale) != 2:
        return False
    vm = np.asarray(video_mask)
    if not bool(vm.all()):
        return False
    bm = np.asarray(block_mask)
    if bm.shape != (N, N):
        return False
    # spot-check causality structure of block_mask (full check is 16M bools)
    idx = np.linspace(0, N - 1, 64).astype(int)
    sub = bm[np.ix_(idx, idx)]
    if not np.array_equal(sub, np.tril(np.ones_like(sub))):
        return False
    return True


def _numpy_reference(x, block_mask, video_mask, qkv_w, qkv_b, proj_w, proj_b,
                     frame_seqlen, sliding_window_size, num_frame_per_block,
                     num_frame_for_scale):
    """Fallback: direct numpy evaluation of the reference semantics."""
    x = np.asarray(x, np.float32)
    b, n, c = x.shape
    dh = c // H
    qkv = (x @ np.asarray(qkv_w).T + np.asarray(qkv_b)).reshape(b, n, 3, H, dh)
    qkv = qkv.transpose(2, 0, 3, 1, 4)
    q, k, v = qkv[0], qkv[1], qkv[2]
    mask = np.asarray(block_mask)[:n, :n][None, None]
    vm = np.asarray(video_mask)[:, None, None, None]
    mask = mask | ~vm
    fs = int(frame_seqlen)
    if int(sliding_window_size) > 0 and fs is not None:
        f = np.arange(n) // fs
        w = int(sliding_window_size) * int(num_frame_per_block)
        sliding = (f[None, :] <= f[:, None]) & (f[None, :] >= f[:, None] - w + 1)
        mask = mask & sliding[None, None]
        if int(num_frame_for_scale) > 0:
            s = int(num_frame_for_scale) * fs
            mask = mask.copy()
            mask[:, :, :, :s] = True
    scores = np.einsum('bhqd,bhkd->bhqk', q, k) * (dh ** -0.5)
    scores = np.where(mask, scores, np.float32(-1e30))
    scores -= scores.max(axis=-1, keepdims=True)
    e = np.exp(scores)
    attn = e / e.sum(axis=-1, keepdims=True)
    o = np.einsum('bhqk,bhkd->bhqd', attn, v)
    o = o.transpose(0, 2, 1, 3).reshape(b, n, c)
    return (o @ np.asarray(proj_w).T + np.asarray(proj_b)).astype(np.float32)


def kernel(x, block_mask, video_mask, qkv_w, qkv_b, proj_w, proj_b,
           frame_seqlen, sliding_window_size, num_frame_per_block,
           num_frame_for_scale):
    if not _check_fixed_params(block_mask, video_mask, frame_seqlen,
                               sliding_window_size, num_frame_per_block,
                               num_frame_for_scale):
        return _numpy_reference(x, block_mask, video_mask, qkv_w, qkv_b,
                                proj_w, proj_b, frame_seqlen,
                                sliding_window_size, num_frame_per_block,
                                num_frame_for_scale)

    from concourse.bass_utils import run_bass_kernel_spmd

    nc = _get_nc()
    in_maps = _host_prep(x, qkv_w, qkv_b, proj_w, proj_b)
    res = run_bass_kernel_spmd(nc, in_maps, core_ids=list(range(NCORES)))
    out = np.empty((N, C), np.float32)
    for i in range(NCORES):
        out[NQ * i:NQ * (i + 1)] = res.results[i]["outT"].T
    return out.reshape(1, N, C)
